# revision 2
# baseline (speedup 1.0000x reference)
# Trainium2 Bass kernel for PlaneFormer-style pairwise-MLP head model.
#
# Strategy (pure data parallel over batch B=64, 8 samples per NeuronCore):
#   reference:  cat[b,i,j] = [emb_i, emb_j, g1, g2]  (B,T,T,4D)
#               h = MLP_heads(cat)  (4 stacked heads, dims 1024->512->256->128->64->4)
#   L0 factorizes: h0[i,j] = relu(X@w0i [i] + X@w0j [j] + g1@w0g1 + g2@w0g2 + b0)
#   On device, per (head, sample):
#     1. ABD = [A; B; D1; D2]  (128 x 512) where A = X@w0i, B = X@w0j,
#        D1 = X@w0g1, D2 = X@w0g2  (PE, writes 4 x 32-row blocks of one PSUM tile)
#     2. h0^T = ABD^T @ S  where S (128 x ncols) is a host-built selection
#        matrix: rows 0-31 one-hot on i, rows 32-63 one-hot on j, rows 64-127
#        carry the masked-mean weights (m1/n0, m2/n1).  K=128 matmul.
#     3. L1..L4 feature-major matmul chain; relu+bias fused into the
#        PSUM->SBUF copies (split between ACT and DVE engines).
#   Heads 0-2 (camera/rot/trans) are masked-mean outputs: only pairs with
#   i < n0 <= 16 and j in [n0, n0+16) can be valid, so they run on a 256-col
#   window; the exact validity mask (as weights /n0*n1) is applied in the
#   final DVE reduction.  Head 3 (plane) needs all 1024 pairs + sigmoid.
#
# Everything model-compute heavy runs on the NeuronCores; the host only
# reshapes inputs, builds the (tiny) selection/mask tensors from num_planes,
# and adds the final b4 bias / sigmoid on the (64,)-sized outputs.

import numpy as np
import ml_dtypes

B, T, D = 64, 32, 256
H = 4
NCORES = 8
SPC = B // NCORES          # samples per core
WIN = 16
CW = WIN * WIN             # 256 window columns (heads 0-2)
CF = T * T                 # 1024 full columns (head 3)
CT = CF + CW               # total S columns per sample
F0, F1, F2, F3, F4 = 512, 256, 128, 64, 4

BF16 = ml_dtypes.bfloat16

_PROG_CACHE = {}
LAST_RESULTS = None


def _build_program():
    """Build the (input-independent) Bass program for one core's 8 samples."""
    import concourse.bass as bass
    import concourse.tile as tile
    from concourse import bacc, mybir

    f32 = mybir.dt.float32
    bf = mybir.dt.bfloat16

    nc = bacc.Bacc("TRN2", target_bir_lowering=False, debug=False,
                   num_devices=NCORES)

    # ---- DRAM parameters (laid out host-side exactly as SBUF wants them) --
    xt_d = nc.dram_tensor("xt", [128, 2, SPC, T], bf, kind="ExternalInput").ap()
    w0_d = nc.dram_tensor("w0s", [128, H, 8, F0], bf, kind="ExternalInput").ap()
    w1_d = nc.dram_tensor("w1s", [128, H, 4, 2, 128], bf, kind="ExternalInput").ap()
    w2_d = nc.dram_tensor("w2s", [128, H, 2, 128], bf, kind="ExternalInput").ap()
    w3_d = nc.dram_tensor("w3s", [128, H, F3], bf, kind="ExternalInput").ap()
    w4_d = nc.dram_tensor("w4s", [F3, H, F4], bf, kind="ExternalInput").ap()
    bias_d = nc.dram_tensor("biases", [128, 33], f32, kind="ExternalInput").ap()
    s_d = nc.dram_tensor("smat", [128, SPC, CT], bf, kind="ExternalInput").ap()
    wv_d = nc.dram_tensor("wv", [F4, SPC, CW], f32, kind="ExternalInput").ap()

    plane_d = nc.dram_tensor("plane", [F4, SPC, CF], f32, kind="ExternalOutput").ap()
    sums_d = nc.dram_tensor("sums", [F4, SPC * 3], f32, kind="ExternalOutput").ap()

    with tile.TileContext(nc) as tc:
        from contextlib import ExitStack
        with ExitStack() as ctx:
            consts = ctx.enter_context(tc.tile_pool(name="consts", bufs=1))
            abpool = ctx.enter_context(tc.tile_pool(name="abpool", bufs=3))
            actp = ctx.enter_context(tc.tile_pool(name="acts", bufs=2))
            outp = ctx.enter_context(tc.tile_pool(name="outs", bufs=1))
            psum = ctx.enter_context(tc.tile_pool(name="ps", bufs=4, space="PSUM"))

            def load_const(ap_d, shape, dt, tag):
                t = consts.tile(shape, dt, tag=tag)
                nc.sync.dma_start(out=t[:], in_=ap_d[:])
                return t

            xt = load_const(xt_d, [128, 2, SPC, T], bf, "c_xt")
            w0 = load_const(w0_d, [128, H, 8, F0], bf, "c_w0")
            w1 = load_const(w1_d, [128, H, 4, 2, 128], bf, "c_w1")
            w2 = load_const(w2_d, [128, H, 2, 128], bf, "c_w2")
            w3 = load_const(w3_d, [128, H, F3], bf, "c_w3")
            w4 = load_const(w4_d, [F3, H, F4], bf, "c_w4")
            bi = load_const(bias_d, [128, 33], f32, "c_bi")
            sm = load_const(s_d, [128, SPC, CT], bf, "c_sm")
            wv = load_const(wv_d, [F4, SPC, CW], f32, "c_wv")

            plane_sb = outp.tile([F4, SPC, CF], f32)
            sums_sb = outp.tile([F4, SPC * 3], f32)

            # engine load-balancing for PSUM->SBUF (relu+bias) copies
            eng_cost = {"act": 0.0, "dve": 0.0}

            def fused_copy(out_ap, in_ap, bias_ap, fd, relu=True):
                c_act = (172 + fd / 2) / 1.2
                c_dve = (120 + fd) / 0.96
                if eng_cost["act"] + c_act <= eng_cost["dve"] + c_dve:
                    eng_cost["act"] += c_act
                    nc.scalar.activation(
                        out=out_ap, in_=in_ap,
                        func=(mybir.ActivationFunctionType.Relu if relu
                              else mybir.ActivationFunctionType.Copy),
                        bias=(bias_ap if bias_ap is not None else 0.0),
                        scale=1.0)
                else:
                    eng_cost["dve"] += c_dve
                    if relu:
                        nc.vector.tensor_scalar(
                            out=out_ap, in0=in_ap,
                            scalar1=bias_ap if bias_ap is not None else 0.0,
                            scalar2=0.0,
                            op0=mybir.AluOpType.add, op1=mybir.AluOpType.max)
                    elif bias_ap is None:
                        nc.vector.tensor_copy(out_ap, in_ap)
                    else:
                        nc.vector.tensor_scalar_add(out_ap, in_ap, bias_ap)

            for s in range(SPC):
                for h in range(H):
                    full = (h == 3)
                    ncols = CF if full else CW
                    colbase = 0 if full else CF
                    nnb = (ncols + 511) // 512  # n-chunks of <=512

                    # ---- ABD: [A; B; D1; D2] (128 x 512) in one PSUM tile
                    abd = psum.tile([128, F0], mybir.dt.float32, tag="ps")
                    for q in range(4):
                        po = 32 * q
                        for kd in range(2):
                            nc.tensor.matmul(
                                abd[po:po + 32, :],
                                lhsT=xt[:, kd, s, :],
                                rhs=w0[:, h, q * 2 + kd, :],
                                start=(kd == 0), stop=(kd == 1),
                                tile_position=(0, po))
                    absb = abpool.tile([128, F0], bf, tag="absb")
                    fused_copy(absb[:, :], abd[:, :], None, F0, relu=False)

                    # ---- S-matmul -> h0 (feature-major, 4 chunks of 128)
                    h0sb = actp.tile([128, 4, ncols], bf, tag="h0")
                    for fc in range(4):
                        ps = psum.tile([128, ncols], mybir.dt.float32, tag="ps")
                        for nb in range(nnb):
                            n0c = nb * 512
                            n1c = min(ncols, n0c + 512)
                            nc.tensor.matmul(
                                ps[:, n0c:n1c],
                                lhsT=absb[:, fc * 128:(fc + 1) * 128],
                                rhs=sm[:, s, colbase + n0c:colbase + n1c],
                                start=True, stop=True)
                        fused_copy(h0sb[:, fc, :], ps[:, :],
                                   bi[:, h * 4 + fc:h * 4 + fc + 1], ncols)

                    # ---- L1: 512 -> 256
                    h1sb = actp.tile([128, 2, ncols], bf, tag="h1")
                    for mf in range(2):
                        ps = psum.tile([128, ncols], mybir.dt.float32, tag="ps")
                        for nb in range(nnb):
                            n0c = nb * 512
                            n1c = min(ncols, n0c + 512)
                            for kd in range(4):
                                nc.tensor.matmul(
                                    ps[:, n0c:n1c],
                                    lhsT=w1[:, h, kd, mf, :],
                                    rhs=h0sb[:, kd, n0c:n1c],
                                    start=(kd == 0), stop=(kd == 3))
                        fused_copy(h1sb[:, mf, :], ps[:, :],
                                   bi[:, 16 + h * 2 + mf:16 + h * 2 + mf + 1],
                                   ncols)

                    # ---- L2: 256 -> 128
                    h2sb = actp.tile([128, ncols], bf, tag="h2")
                    ps = psum.tile([128, ncols], mybir.dt.float32, tag="ps")
                    for nb in range(nnb):
                        n0c = nb * 512
                        n1c = min(ncols, n0c + 512)
                        for kd in range(2):
                            nc.tensor.matmul(
                                ps[:, n0c:n1c],
                                lhsT=w2[:, h, kd, :],
                                rhs=h1sb[:, kd, n0c:n1c],
                                start=(kd == 0), stop=(kd == 1))
                    fused_copy(h2sb[:, :], ps[:, :], bi[:, 24 + h:25 + h], ncols)

                    # ---- L3: 128 -> 64
                    h3sb = actp.tile([F3, ncols], bf, tag="h3")
                    ps = psum.tile([F3, ncols], mybir.dt.float32, tag="ps")
                    for nb in range(nnb):
                        n0c = nb * 512
                        n1c = min(ncols, n0c + 512)
                        nc.tensor.matmul(
                            ps[:, n0c:n1c], lhsT=w3[:, h, :],
                            rhs=h2sb[:, n0c:n1c], start=True, stop=True)
                    fused_copy(h3sb[:, :], ps[:, :],
                               bi[0:F3, 28 + h:29 + h], ncols)

                    # ---- L4: 64 -> 4 (+ final reductions)
                    ps4 = psum.tile([F4, ncols], mybir.dt.float32, tag="ps")
                    for nb in range(nnb):
                        n0c = nb * 512
                        n1c = min(ncols, n0c + 512)
                        nc.tensor.matmul(
                            ps4[:, n0c:n1c], lhsT=w4[:, h, :],
                            rhs=h3sb[:, n0c:n1c], start=True, stop=True)
                    if full:
                        # plane head: sigmoid(h4 + b4[3]) on all 4 rows
                        # (only row 0 is consumed host-side)
                        nc.scalar.activation(
                            out=plane_sb[:, s, :], in_=ps4[:, :],
                            func=mybir.ActivationFunctionType.Sigmoid,
                            bias=bi[0:F4, 32:33], scale=1.0)
                    else:
                        # masked mean: accum_out[c] = sum_ij h4[c,ij]*wv[ij]
                        scratch = abpool.tile([F4, CW], mybir.dt.float32,
                                              tag="scr")
                        nc.vector.scalar_tensor_tensor(
                            out=scratch[:, :], in0=ps4[:, :], scalar=1.0,
                            in1=wv[:, s, :],
                            op0=mybir.AluOpType.mult,
                            op1=mybir.AluOpType.mult,
                            accum_out=sums_sb[:, s * 3 + h:s * 3 + h + 1])

            nc.sync.dma_start(out=plane_d[:], in_=plane_sb[:])
            nc.sync.dma_start(out=sums_d[:], in_=sums_sb[:])

    nc.compile()
    return nc


def _host_prep(emb, num_planes, w0, b0, w1, b1, w2, b2, w3, b3, w4, b4):
    """Build per-core input maps (all layouts partition-major for 1-DMA loads)."""
    emb = np.asarray(emb, np.float32)
    npl = np.asarray(num_planes).astype(np.int64)
    n0 = npl[:, 0]
    n1 = npl[:, 1]
    assert n0.min() >= 1 and n1.min() >= 1 and n0.max() <= 16 and n1.max() <= 16

    idx = np.arange(T)
    m1 = idx[None, :] < n0[:, None]                       # (B,T)
    in2 = (idx[None, :] >= n0[:, None]) & (idx[None, :] < (n0 + n1)[:, None])
    mw1 = (m1 / n0[:, None]).astype(np.float32)
    mw2 = (in2 / n1[:, None]).astype(np.float32)

    # xt: [128, kd, s, i] = emb[s, i, kd*128+p]
    xt = np.ascontiguousarray(
        emb.transpose(2, 0, 1).reshape(2, 128, B, T).transpose(1, 0, 2, 3)
    ).astype(BF16)

    w0s = np.ascontiguousarray(
        np.asarray(w0, np.float32).reshape(H, 8, 128, F0).transpose(2, 0, 1, 3)
    ).astype(BF16)
    w1s = np.ascontiguousarray(
        np.asarray(w1, np.float32).reshape(H, 4, 128, 2, 128)
        .transpose(2, 0, 1, 3, 4)).astype(BF16)
    w2s = np.ascontiguousarray(
        np.asarray(w2, np.float32).reshape(H, 2, 128, 128).transpose(2, 0, 1, 3)
    ).astype(BF16)
    w3s = np.ascontiguousarray(
        np.asarray(w3, np.float32).transpose(1, 0, 2)).astype(BF16)
    w4s = np.ascontiguousarray(
        np.asarray(w4, np.float32).transpose(1, 0, 2)).astype(BF16)

    biases = np.zeros((128, 33), np.float32)
    biases[:, 0:16] = np.asarray(b0, np.float32).reshape(H, 4, 128) \
        .transpose(2, 0, 1).reshape(128, 16)
    biases[:, 16:24] = np.asarray(b1, np.float32).reshape(H, 2, 128) \
        .transpose(2, 0, 1).reshape(128, 8)
    biases[:, 24:28] = np.asarray(b2, np.float32).T
    biases[0:F3, 28:32] = np.asarray(b3, np.float32).T
    biases[0:F4, 32] = np.asarray(b4, np.float32)[3]

    # selection matrix S: (B, 128, CT)
    S = np.zeros((B, 128, CT), np.float32)
    cols = np.arange(CF)
    iF, jF = cols // T, cols % T
    S[:, iF, cols] = 1.0
    S[:, 32 + jF, cols] = 1.0
    cw = np.arange(CW)
    iw, jw = cw // WIN, cw % WIN
    S[:, iw, CF + cw] = 1.0
    for b in range(B):
        S[b, 32 + n0[b] + jw, CF + cw] = 1.0
    S[:, 64:96, :] = mw1[:, :, None]
    S[:, 96:128, :] = mw2[:, :, None]
    S = S.astype(BF16)

    # window mask weights (valid / (n0*n1)), replicated on 4 partitions
    pf = (n0 * n1).astype(np.float32)
    wvw = ((iw[None, :] < n0[:, None]) & (jw[None, :] < n1[:, None])) \
        / pf[:, None]                                      # (B, CW)
    wvw = np.broadcast_to(wvw[:, None, :].astype(np.float32), (B, F4, CW))

    in_maps = []
    for c in range(NCORES):
        sl = slice(c * SPC, (c + 1) * SPC)
        in_maps.append({
            "xt": np.ascontiguousarray(xt[:, :, sl, :]),
            "w0s": w0s, "w1s": w1s, "w2s": w2s, "w3s": w3s, "w4s": w4s,
            "biases": biases,
            "smat": np.ascontiguousarray(S[sl].transpose(1, 0, 2)),
            "wv": np.ascontiguousarray(wvw[sl].transpose(1, 0, 2)),
        })
    meta = dict(m1=m1, in2=in2, b4=np.asarray(b4, np.float32))
    return in_maps, meta


def _host_post(results, meta):
    b4 = meta["b4"]
    cam = np.zeros(B, np.float32)
    rot = np.zeros((B, 4), np.float32)
    trn = np.zeros((B, 3), np.float32)
    plane = np.zeros((B, T, T), np.float32)
    for c in range(NCORES):
        pl = results[c]["plane"]      # (4, SPC, CF)
        sm = results[c]["sums"]       # (4, SPC*3)
        for s in range(SPC):
            b = c * SPC + s
            plane[b] = pl[0, s].reshape(T, T)
            cam[b] = sm[0, s * 3 + 0] + b4[0, 0]
            rot[b] = sm[:, s * 3 + 1] + b4[1]
            trn[b] = sm[0:3, s * 3 + 2] + b4[2, :3]
    cam = (1.0 / (1.0 + np.exp(-cam.astype(np.float64)))).astype(np.float32)
    valid = meta["m1"][:, :, None] & meta["in2"][:, None, :]
    return cam, rot, trn, plane, valid


def kernel(emb, num_planes, w0, b0, w1, b1, w2, b2, w3, b3, w4, b4,
           _trace=False):
    global LAST_RESULTS
    from concourse.bass_utils import run_bass_kernel_spmd

    if "prog" not in _PROG_CACHE:
        _PROG_CACHE["prog"] = _build_program()
    nc = _PROG_CACHE["prog"]

    in_maps, meta = _host_prep(emb, num_planes, w0, b0, w1, b1,
                               w2, b2, w3, b3, w4, b4)
    res = run_bass_kernel_spmd(nc, in_maps, list(range(NCORES)),
                               trace=_trace)
    LAST_RESULTS = res
    return _host_post(res.results, meta)
